# revision 22
# baseline (speedup 1.0000x reference)
"""MegaCRN forward on 8 Trainium2 cores, data-parallel over batch.

Per core: B=8 batch shard. All matmuls run in bf16 (same 1 col/cycle PE
stream rate as f32r, but FWL-fast weight loads, 2x cheaper DVE elementwise,
and half the weight-DMA bytes). The ENCODER graph convs additionally run in
fp8e4m3 DoubleRow (contraction 256 rows/matmul -> ~1.5x): encoder fp8 noise
washes out through the memory-attention bottleneck and 12 decoder steps
(simulated end-to-end rel err 1.12e-2 vs 1.07e-2 for pure bf16), while the
same trick on the decoder fails the 2e-2 budget (2.9e-2).

Layouts:
  node layout : (N partitions, batch*chan free)  -- graph-conv contraction
  chan layout : (chan partitions, N free)        -- channel projections
Chebyshev supports stored TRANSPOSED (AT_j = S_j^T) so the graph conv yields
chan-layout Y^T = (S_j @ h)^T via matmul(lhsT=H_node_slice, rhs=AT_j).
The fp8 support copies are scaled by S_SUP=32768 (entries ~1/512 would be
subnormal otherwise); the T2 fp8 supports hold plain 2A^2 -- the exact -I
term is folded into the h-term weights host-side, and 1/S_SUP is folded into
the conv-term projection weights. Encoder node-layout state (|h|<=1) casts
to fp8 unscaled during the usual transpose-psum evacuation.

The adaptive supports and ALL x-features (identity + the 4 support convs +
bias lane) depend only on weights/inputs and are computed host-side, shipped
as ATB (bf16), AT8H (fp8), and xftb; the device runs no startup build.

Output is the raw bf16 GO_ALL tile (98KB/core); kernel() converts host-side.
"""
import numpy as np
import ml_dtypes

import concourse.bass as bass
import concourse.tile as tile
from concourse import bacc, mybir
from concourse.bass_utils import run_bass_kernel_spmd

F32 = mybir.dt.float32
F32R = mybir.dt.float32r
I8 = mybir.dt.int8
AF = mybir.ActivationFunctionType
AX = mybir.AxisListType
BF16 = mybir.dt.bfloat16

N = 512
B = 8          # per-core batch
L = 12
H = 12
R = 64         # rnn units
D = 128        # dec_dim
MN = 20        # mem_num
MD = 64        # mem_dim
NT = 4         # node tiles (N/128)
EVAC_MOD = 2   # 1 of EVAC_MOD evacuations goes to ScalarE
YB_BUFS = 6
TRZ_BUFS = 8
PSA_BUFS = 2
PSC_BUFS = 2
ENC_STEPS = L
DEC_STEPS = H
DO_ATT = True
ENC_SPREAD = True
DEC_SPREAD = False
DEC_CONV_SPREAD = True
YEVAC_DVE = False
ATT_BUFS = 1
NODE_BUFS = 1
SPREAD2 = False
PHASE_MAJOR = True
GSZ = 4
ENC_PHASE_MAJOR = True
ENC_GSZ = 4
S_SUP = 32768.0   # fp8 scale on the adaptive supports (entries ~1/512)
F8 = mybir.dt.float8e4
DR = mybir.MatmulPerfMode.DoubleRow


def _pack_weights(Memory, Wq, We1, We2, egW, egb, euW, eub, dgW, dgb, duW, dub, pW, pb):
    W = {}
    W["Mem"] = Memory                                        # (20, 64)
    # adaptive supports: depend only on weights -> computed host-side in f32.
    e1 = We1 @ Memory
    e2 = We2 @ Memory
    def _smax(a):
        ex = np.exp(a - a.max(-1, keepdims=True))
        return ex / ex.sum(-1, keepdims=True)
    g1 = _smax(np.maximum(e1 @ e2.T, 0.0))
    g2 = _smax(np.maximum(e2 @ e1.T, 0.0))
    eye = np.eye(g1.shape[0], dtype=np.float32)
    sup_b = [g1, 2.0 * g1 @ g1 - eye, g2, 2.0 * g2 @ g2 - eye]
    sup_8 = [g1, 2.0 * g1 @ g1, g2, 2.0 * g2 @ g2]   # T2 without -I (folded)
    # device layout: AT[p, kt, n] = S[n, kt*128+p] (transposed support)
    W["ATB"] = np.concatenate([np.ascontiguousarray(sj.T) for sj in sup_b], axis=0)
    W["AT8H"] = np.concatenate(
        [np.clip(sj.T * S_SUP, -240.0, 240.0) for sj in sup_8], axis=0)
    MemTD = np.zeros((128, 40), np.float32)
    MemTD[:64, :20] = Memory.T
    MemTD[64:, 20:] = Memory.T
    W["MemTD"] = MemTD
    WqD = np.zeros((128, 128), np.float32)
    WqD[:64, :64] = Wq
    WqD[64:, 64:] = Wq
    W["WqD"] = WqD
    Pq = np.concatenate([np.arange(64, 128), np.arange(0, 64)])
    W["pW"] = pW.reshape(128, 1)[Pq]
    # masked copies of pW: col b of block b holds pW so 8 accumulating
    # matmuls stack the per-batch go projections into one (8, N) psum
    pWm = np.zeros((128, 8 * 8), np.float32)
    for b in range(8):
        pWm[:, b * 8 + b] = W["pW"][:, 0]
    W["pWm"] = pWm
    W["ident"] = np.eye(128, dtype=np.float32)

    # encoder: blocks of 65 rows [x(1), h(64)], order (I, g1, T2g1, I, g2, T2g2)
    def eb(Wm, k):
        return Wm[k * 65 + 1: k * 65 + 65]

    # encoder convs run in fp8 DoubleRow with the T2 supports stored as plain
    # 2A^2 (no -I, which would overflow the fp8 support scale): the -I term is
    # exact and folds into the h-term weights. Conv-term weight slots absorb
    # the 1/S_SUP dequant of the fp8 support scale.
    Wh = [eb(egW, 0) + eb(egW, 3) - eb(egW, 2) - eb(egW, 5),
          eb(egW, 1), eb(egW, 2), eb(egW, 4), eb(egW, 5)]
    Whu = [eb(euW, 0) + eb(euW, 3) - eb(euW, 2) - eb(euW, 5),
           eb(euW, 1), eb(euW, 2), eb(euW, 4), eb(euW, 5)]
    wez = np.zeros((5, 128, 128), np.float32)
    wer = np.zeros((5, 128, 128), np.float32)
    weu = np.zeros((5, 128, 128), np.float32)
    for j in range(5):
        sc = 1.0 if j == 0 else 1.0 / S_SUP
        for h2 in range(2):
            s = slice(h2 * 64, h2 * 64 + 64)
            wez[j][s, s] = Wh[j][:, 0:64] * sc
            wer[j][s, s] = Wh[j][:, 64:128] * sc
            weu[j][s, s] = Whu[j] * sc
    W["WEZ"], W["WER"], W["WEU"] = wez, wer, weu

    exg = [egW[0] + egW[195], egW[65], egW[130], egW[260], egW[325]]
    exu = [euW[0] + euW[195], euW[65], euW[130], euW[260], euW[325]]
    xfw = np.zeros((4, 3, 48, 128), np.float32)
    for p in range(4):
        for bl in range(2):
            b = 2 * p + bl
            cs = slice(bl * 64, bl * 64 + 64)
            for s in range(5):
                xfw[p, 0, s * 8 + b, cs] = exg[s][0:64]
                xfw[p, 1, s * 8 + b, cs] = exg[s][64:128]
                xfw[p, 2, s * 8 + b, cs] = exu[s]
            xfw[p, 0, 40, cs] = egb[0:64]
            xfw[p, 1, 40, cs] = egb[64:128]
            xfw[p, 2, 40, cs] = eub
    W["XFW"] = xfw

    # decoder: blocks of 134 rows [go(1), yc(5), h(128)]
    def db(Wm, k):
        return Wm[k * 134 + 6: k * 134 + 134]

    P = np.concatenate([np.arange(64, 128), np.arange(0, 64)])
    def ph_rows(Wm):   # permute h-input rows
        return Wm[P]
    def ph_zr(Wm):     # permute gate out cols (z block, r block)
        Wm = Wm.copy()
        Wm[:, 0:128] = Wm[:, 0:128][:, P]
        Wm[:, 128:256] = Wm[:, 128:256][:, P]
        return Wm
    def ph_u(Wm):      # permute update out cols
        return Wm[:, P] if Wm.ndim == 2 else Wm[P]
    Wdh = [ph_zr(ph_rows(db(dgW, 0) + db(dgW, 3))), ph_zr(ph_rows(db(dgW, 1))),
           ph_zr(ph_rows(db(dgW, 2))), ph_zr(ph_rows(db(dgW, 4))),
           ph_zr(ph_rows(db(dgW, 5)))]
    Wduh = [ph_u(ph_rows(db(duW, 0) + db(duW, 3))), ph_u(ph_rows(db(duW, 1))),
            ph_u(ph_rows(db(duW, 2))), ph_u(ph_rows(db(duW, 4))),
            ph_u(ph_rows(db(duW, 5)))]
    wdg = np.zeros((5, 2, 128, 128), np.float32)
    wdu = np.zeros((5, 128, 128), np.float32)
    for j in range(5):
        wdg[j, 0] = Wdh[j][:, 0:128]
        wdg[j, 1] = Wdh[j][:, 128:256]
        wdu[j] = Wduh[j]
    W["WDG"], W["WDU"] = wdg, wdu

    def pv_zr(v):
        v = v.copy()
        v[0:128] = v[0:128][P]
        v[128:256] = v[128:256][P]
        return v
    dgo = [pv_zr(dgW[0] + dgW[3 * 134]), pv_zr(dgW[134]), pv_zr(dgW[2 * 134]),
           pv_zr(dgW[4 * 134]), pv_zr(dgW[5 * 134])]
    duo = [(duW[0] + duW[3 * 134])[P], duW[134][P], duW[2 * 134][P],
           duW[4 * 134][P], duW[5 * 134][P]]
    xdw = np.zeros((8, 3, 48, 128), np.float32)
    for b in range(8):
        for s in range(5):
            xdw[b, 0, s * 8 + b] = dgo[s][0:128]
            xdw[b, 1, s * 8 + b] = dgo[s][128:256]
            xdw[b, 2, s * 8 + b] = duo[s]
    W["XDW"] = xdw

    wycg = np.zeros((8, 256), np.float32)
    wycu = np.zeros((8, 128), np.float32)
    for k in range(6):
        wycg[0:5] += dgW[k * 134 + 1: k * 134 + 6]
        wycu[0:5] += duW[k * 134 + 1: k * 134 + 6]
    wycg[5] = dgb
    wycu[5] = dub
    wycg[:, 0:128] = wycg[:, 0:128][:, P]
    wycg[:, 128:256] = wycg[:, 128:256][:, P]
    wycu = wycu[:, P]
    W["WYCG"], W["WYCU"] = wycg, wycu
    W["pb"] = float(np.asarray(pb).reshape(-1)[0])
    return W


def _emit(nc, tc, dram, out8_d, pb):
    ctxs = []

    def pool(name, bufs, space="SBUF"):
        p = tc.tile_pool(name=name, bufs=bufs, space=space)
        ctxs.append(p)
        return p.__enter__()

    wp = pool("wts", 1)
    psA = pool("psA", PSA_BUFS, "PSUM")
    psE = pool("psE", 1, "PSUM")
    psD = pool("psD", 1, "PSUM")
    psB = pool("psB", 2, "PSUM")
    psC = pool("psC", PSC_BUFS, "PSUM")
    big = pool("big", 1)          # AT supports, XFT
    node = pool("node", NODE_BUFS)        # node-layout state (G1/G2 early, H later)
    hTp = pool("hTp", 2)          # enc chan-layout h (pairs packed in one tile)
    hdTp = pool("hdTp", 9)       # dec chan-layout h per batch
    zhp = pool("zhp", 1)          # ZH node layout
    yb = pool("yb", YB_BUFS)            # evacuated Y^T tiles (also qT / X staging)
    trz = pool("trz", TRZ_BUFS)          # transient chan tiles (z, r, hc, zh, tmp)
    xdp = pool("xdp", 2)          # dec dynamic x features
    sm = pool("sm", 2)            # small stuff

    evac_ct = [0]

    def evac(dst, src, scale=None, dve=False):
        i = evac_ct[0]
        evac_ct[0] += 1
        if scale is not None:
            nc.scalar.activation(dst, src, AF.Copy, scale=scale)
        elif (dve and YEVAC_DVE) or i % EVAC_MOD != EVAC_MOD - 1:
            nc.vector.tensor_copy(dst, src)
        else:
            nc.scalar.activation(dst, src, AF.Copy)

    def load(name, shape, p=wp, tag=None):
        t = p.tile(list(shape), BF16, tag=tag or name)
        nc.sync.dma_start(t[:], dram[name][:])
        return t

    ident = load("ident", (128, 128))

    def load_w3(name, n0):
        t = wp.tile([128, n0, 128], BF16, tag=name)
        nc.sync.dma_start(t[:], dram[name][:].rearrange("j k m -> k j m"))
        return t

    # DMA order = first-use order: encoder tensors (XFT/XFW/WEZ..., AT8)
    # first so step 0/1 start ASAP; attention tensors next; the 2MB bf16
    # supports (decoder-only) last.
    XFT = big.tile([48, L, N], BF16, tag="XFT")
    nc.sync.dma_start(XFT[:], dram["xftb"][:].rearrange("o (t n) -> o t n", t=L))
    XFWt = wp.tile([48, 4, 3, 128], BF16, tag="XFW")
    nc.sync.dma_start(XFWt[:], dram["XFW"][:].rearrange("p o k m -> k p o m"))
    WEZt = load_w3("WEZ", 5)
    WERt = load_w3("WER", 5)
    WEUt = load_w3("WEU", 5)

    # ---------------- adaptive supports (host-computed, DMA-loaded) ----------
    # ATb: real-valued bf16 supports (T2 slots include the -I) for the decoder
    # and the x-feature convs. AT8: fp8 supports scaled by S_SUP (T2 slots hold
    # plain 2A^2) for the encoder DoubleRow convs. Both depend only on weights,
    # so the softmax/Chebyshev build runs on the host.
    AT = [big.tile([128, NT, N], BF16, tag=f"AT{j}", name=f"AT{j}") for j in range(4)]
    AT8 = [big.tile([128, NT, N], F8, tag=f"AT8{j}", name=f"AT8{j}") for j in range(4)]
    for j in range(4):
        nc.sync.dma_start(
            AT8[j][:], dram["AT8H"][j * N:(j + 1) * N, :]
            .rearrange("(kt kp) n -> kp kt n", kp=128))

    Mem = load("Mem", (MN, MD))
    MemTD = load("MemTD", (128, 2 * MN))
    WqD = load("WqD", (128, 128))
    pWm = load("pWm", (128, 64))
    for j in range(4):
        nc.sync.dma_start(
            AT[j][:], dram["ATB"][j * N:(j + 1) * N, :]
            .rearrange("(kt kp) n -> kp kt n", kp=128))

    ycv = load("ycv", (8, B * H))
    WYCG = load("WYCG", (8, 256))
    WYCU = load("WYCU", (8, 128))
    ycb = []
    for i in range(3):
        lhs = WYCG[:, i * 128:(i + 1) * 128] if i < 2 else WYCU[:]
        ps = psC.tile([128, B * H], F32, tag="c")
        nc.tensor.matmul(ps[:], lhsT=lhs, rhs=ycv[:], start=True, stop=True)
        t = sm.tile([128, B * H], F32, tag=f"ycb{i}", bufs=1, name=f"ycb{i}")
        evac(t[:], ps[:])
        ycb.append(t)
    ycZ, ycR, ycHC = ycb

    # ---------------- encoder ----------------
    # h0 = 0, so step 0 needs no node-layout state: conv(0) = 0 and
    # z*h = 0 exactly -- the whole step collapses to the x-feature matmuls.
    Hn = None
    hT = hTp.tile([128, 4, N], BF16, tag="hT")
    zsc = trz.tile([128, N], BF16, tag="t", name="zsc")
    nc.vector.memset(zsc[:], 0.0)
    for kt in range(NT):
        nc.vector.tensor_copy(hT[:, kt, :], zsc[:])

    for t in range(ENC_STEPS):
        new_hT = hTp.tile([128, 4, N], BF16, tag="hT")
        # encoder node-layout state is fp8: consumed only by DoubleRow convs
        ZHn = (zhp.tile([128, NT, B * D], F8, tag="zh8", name="ZHn")
               if t > 0 else None)
        # the final step's node-layout state is never consumed
        newHn = (node.tile([128, NT, B * D], F8, tag="node8", name="newHn")
                 if t < ENC_STEPS - 1 else None)
        def ephase_a(p):
            psl = slice(p * 128, (p + 1) * 128)
            if t == 0:
                # h = 0: conv terms and z-gate vanish; only r is needed
                ps = psB.tile([128, N], F32, tag="b", name="ps_r0")
                nc.tensor.matmul(ps[:], lhsT=XFWt[:, p, 1, :], rhs=XFT[:, 0, :],
                                 start=True, stop=True)
                r = trz.tile([128, N], BF16, tag="t", name="r")
                nc.scalar.activation(r[:], ps[:], AF.Sigmoid)
                return r, None
            # gate graph conv for this pair
            sbY = []
            for j in range(4):
                pool_j = (psD if j == 3 else psE if (j == 2 and SPREAD2) else psA) \
                    if ENC_SPREAD else psA
                ps = pool_j.tile(
                    [128, N], F32,
                    tag="d" if pool_j is psD else "e" if pool_j is psE else "a",
                    name="psYe")
                for g2 in range(2):
                    nc.tensor.matmul(ps[:], lhsT=Hn[:, 2 * g2:2 * g2 + 2, psl],
                                     rhs=AT8[j][:, 2 * g2:2 * g2 + 2, :],
                                     start=(g2 == 0), stop=(g2 == 1), perf_mode=DR)
                sb = yb.tile([128, N], BF16, tag="yb")
                evac(sb[:], ps[:], dve=True)
                sbY.append(sb)
            # gate projection + sigmoid
            zr = []
            for oi, Wt in ((0, WEZt), (1, WERt)):
                ps = psB.tile([128, N], F32, tag="b")
                nc.tensor.matmul(ps[:], lhsT=XFWt[:, p, oi, :], rhs=XFT[:, t, :],
                                 start=True, stop=False)
                nc.tensor.matmul(ps[:], lhsT=Wt[:, 0, :], rhs=hT[:, p, :],
                                 start=False, stop=False)
                for j in range(4):
                    nc.tensor.matmul(ps[:], lhsT=Wt[:, j + 1, :], rhs=sbY[j][:],
                                     start=False, stop=(j == 3))
                zr.append(ps)
            z = trz.tile([128, N], BF16, tag="t", name="z")
            nc.scalar.activation(z[:], zr[0][:], AF.Sigmoid)
            r = trz.tile([128, N], BF16, tag="t", name="r")
            nc.scalar.activation(r[:], zr[1][:], AF.Sigmoid)
            zht = trz.tile([128, N], BF16, tag="t", name="zh")
            nc.vector.tensor_mul(zht[:], z[:], hT[:, p, :])
            # zh -> node layout (cols of this pair)
            for kt in range(NT):
                pst = psC.tile([128, 128], BF16, tag="c")
                nc.tensor.transpose(pst[:], zht[:, kt * 128:(kt + 1) * 128], ident[:])
                evac(ZHn[:, kt, psl], pst[:])
            return r, zht

        def ephase_b(p, r, zht):
            psl = slice(p * 128, (p + 1) * 128)
            if t == 0:
                ps = psB.tile([128, N], F32, tag="b", name="ps_u0")
                nc.tensor.matmul(ps[:], lhsT=XFWt[:, p, 2, :], rhs=XFT[:, 0, :],
                                 start=True, stop=True)
                hc = trz.tile([128, N], BF16, tag="t", name="hc")
                nc.scalar.activation(hc[:], ps[:], AF.Tanh)
                tmp = trz.tile([128, N], BF16, tag="t", name="tmp")
                nc.vector.tensor_sub(tmp[:], hT[:, p, :], hc[:])
                nc.vector.tensor_mul(tmp[:], r[:], tmp[:])
                nc.vector.tensor_add(new_hT[:, p, :], hc[:], tmp[:])
                if newHn is not None:
                    for kt in range(NT):
                        pst = psE.tile([128, 128], BF16, tag="e", name="pst_h")
                        nc.tensor.transpose(
                            pst[:], new_hT[:, p, kt * 128:(kt + 1) * 128],
                            ident[:])
                        evac(newHn[:, kt, psl], pst[:])
                return
            # update graph conv on zh
            sbU = []
            for j in range(4):
                pool_j = psD if (ENC_SPREAD and j == 3) else psA
                ps = pool_j.tile([128, N], F32, tag="d" if pool_j is psD else "a",
                                 name="psUe")
                for g2 in range(2):
                    nc.tensor.matmul(ps[:], lhsT=ZHn[:, 2 * g2:2 * g2 + 2, psl],
                                     rhs=AT8[j][:, 2 * g2:2 * g2 + 2, :],
                                     start=(g2 == 0), stop=(g2 == 1), perf_mode=DR)
                sb = yb.tile([128, N], BF16, tag="yb")
                evac(sb[:], ps[:], dve=True)
                sbU.append(sb)
            ps = psB.tile([128, N], F32, tag="b")
            nc.tensor.matmul(ps[:], lhsT=XFWt[:, p, 2, :], rhs=XFT[:, t, :],
                             start=True, stop=False)
            nc.tensor.matmul(ps[:], lhsT=WEUt[:, 0, :], rhs=zht[:],
                             start=False, stop=False)
            for j in range(4):
                nc.tensor.matmul(ps[:], lhsT=WEUt[:, j + 1, :], rhs=sbU[j][:],
                                 start=False, stop=(j == 3))
            hc = trz.tile([128, N], BF16, tag="t", name="hc")
            nc.scalar.activation(hc[:], ps[:], AF.Tanh)
            tmp = trz.tile([128, N], BF16, tag="t", name="tmp")
            nc.vector.tensor_sub(tmp[:], hT[:, p, :], hc[:])
            nc.vector.tensor_mul(tmp[:], r[:], tmp[:])
            nc.vector.tensor_add(new_hT[:, p, :], hc[:], tmp[:])
            # h -> node layout (skipped on the last step: dead state)
            if newHn is not None:
                for kt in range(NT):
                    pst = psE.tile([128, 128], BF16, tag="e", name="pst_h")
                    nc.tensor.transpose(pst[:],
                                        new_hT[:, p, kt * 128:(kt + 1) * 128],
                                        ident[:])
                    evac(newHn[:, kt, psl], pst[:])

        if ENC_PHASE_MAJOR:
            for g in range(4 // ENC_GSZ):
                grp = list(range(g * ENC_GSZ, (g + 1) * ENC_GSZ))
                astate = [ephase_a(p) for p in grp]
                for pi, p in enumerate(grp):
                    ephase_b(p, *astate[pi])
        else:
            for p in range(4):
                ephase_b(p, *ephase_a(p))
        hT = new_hT
        Hn = newHn

    # ---------------- memory attention ----------------
    qT = []
    for p in range(4):
        ps = psA.tile([128, N], F32, tag="a")
        nc.tensor.matmul(ps[:], lhsT=WqD[:], rhs=hT[:, p, :], start=True, stop=True)
        q = yb.tile([128, N], BF16, tag="yb", name="qT")
        evac(q[:], ps[:])
        qT.append(q)
    hdT = []
    # the 32 per-(pair, node-tile, half) softmax chains are tiny latency-bound
    # ops: spread the logit psums over 3 pools and run the transient tiles
    # 6 deep so chains from different pairs pipeline instead of serializing
    for p in range(4):
        pool_t = psC if p % 2 == 0 else psA
        attT_ps = [pool_t.tile([MN, N], BF16,
                               tag="c" if pool_t is psC else "a",
                               name=f"attTps{h2}")
                   for h2 in range(2)]
        for nb in range(NT):
            pool_l = (psB, psD, psE)[(p * NT + nb) % 3]
            ps = pool_l.tile([128, 2 * MN], F32,
                             tag="b" if pool_l is psB else
                             "d" if pool_l is psD else "e",
                             name="att_lg")
            nc.tensor.matmul(ps[:], lhsT=qT[p][:, nb * 128:(nb + 1) * 128],
                             rhs=MemTD[:], start=True, stop=True)
            for h2 in range(2):
                psl2 = ps[:, h2 * MN:(h2 + 1) * MN]
                # attention logits are O(1): exp is overflow-safe unshifted
                ex = sm.tile([128, MN], F32, tag="aex", bufs=6, name="aex")
                nc.scalar.activation(ex[:], psl2, AF.Exp)
                ssum = sm.tile([128, 1], F32, tag="assum", bufs=6, name="assum")
                nc.vector.reduce_sum(ssum[:], ex[:], AX.X)
                rcp = sm.tile([128, 1], F32, tag="arcp", bufs=6, name="arcp")
                nc.vector.reciprocal(rcp[:], ssum[:])
                att = sm.tile([128, MN], BF16, tag="aatt", bufs=6, name="aatt")
                nc.vector.tensor_scalar_mul(att[:], ex[:], rcp[:])
                nc.tensor.transpose(attT_ps[h2][:, nb * 128:(nb + 1) * 128],
                                    att[:], ident[:])
        for h2 in range(2):
            b = 2 * p + h2
            attT = sm.tile([MN, N], BF16, tag="attT", bufs=2, name="attT")
            evac(attT[:], attT_ps[h2][:])
            ps = psB.tile([MD, N], F32, tag="b")
            nc.tensor.matmul(ps[:], lhsT=Mem[:], rhs=attT[:],
                             start=True, stop=True)
            hh = hdTp.tile([128, N], BF16, tag="hdT", name="hh")
            evac(hh[0:64, :], ps[:])
            nc.sync.dma_start(hh[64:128, :], hT[h2 * 64:(h2 + 1) * 64, p, :])
            hdT.append(hh)

    Hdn = node.tile([128, NT, B * D], BF16, tag="node")
    for kt in range(NT):
        for g2 in range(2):
            ps = psE.tile([128, N], BF16, tag="e", name="ps_hd")
            for bi in range(4):
                b = g2 * 4 + bi
                nc.tensor.transpose(ps[:, bi * 128:(bi + 1) * 128],
                                    hdT[b][:, kt * 128:(kt + 1) * 128], ident[:])
            evac(Hdn[:, kt, g2 * N:(g2 + 1) * N], ps[:])

    # ---------------- decoder ----------------
    # decoder weights reuse the encoder weight slots (enc weights dead by now)
    WDGt = wp.tile([128, 5, 2, 128], BF16, tag="WEZ", name="WDGt")
    nc.sync.dma_start(WDGt[:], dram["WDG"][:].rearrange("j t k m -> k j t m"))
    WDUt = wp.tile([128, 5, 128], BF16, tag="WER", name="WDUt")
    nc.sync.dma_start(WDUt[:], dram["WDU"][:].rearrange("j k m -> k j m"))
    XDWt = wp.tile([48, 8, 3, 128], BF16, tag="XFW", name="XDWt")
    nc.sync.dma_start(XDWt[:], dram["XDW"][:].rearrange("b o k m -> k b o m"))
    # go0 = 0, so step 0's x-feature matmuls vanish: no Xdyn until step 1
    Xdyn = None
    # kt-major free layout: the psgt transpose results evacuate in ONE
    # contiguous copy per step, and the Xdyn conv lhsT slices are contiguous
    GO_ALL = sm.tile([128, DEC_STEPS, NT, B], BF16, tag="go_all", bufs=1,
                     name="GO_ALL")

    for t in range(DEC_STEPS):
        new_hdT = []
        ZHn = zhp.tile([128, NT, B * D], BF16, tag="zh_n")
        newHdn = (node.tile([128, NT, B * D], BF16, tag="node", name="newHdn")
                  if t < DEC_STEPS - 1 else None)
        def phase_a(b):
            bsl = slice(b * 128, (b + 1) * 128)
            cur = hdT[b]
            sbY = []
            for j in range(4):
                pool_j = (psE if j == 3 else psD if j == 2 else psA) \
                    if DEC_CONV_SPREAD else psA
                ps = pool_j.tile(
                    [128, N], F32,
                    tag="e" if pool_j is psE else "d" if pool_j is psD else "a",
                    name="psYd")
                for kt in range(NT):
                    nc.tensor.matmul(ps[:], lhsT=Hdn[:, kt, bsl], rhs=AT[j][:, kt, :],
                                     start=(kt == 0), stop=(kt == 3))
                sb = yb.tile([128, N], BF16, tag="yb", name="sbY")
                evac(sb[:], ps[:], dve=True)
                sbY.append(sb)
            zr = []
            for mt in range(2):
                ps = psB.tile([128, N], F32, tag="b")
                if Xdyn is not None:
                    nc.tensor.matmul(ps[:], lhsT=XDWt[0:40, b, mt, :],
                                     rhs=Xdyn[0:40, :], start=True, stop=False)
                nc.tensor.matmul(ps[:], lhsT=WDGt[:, 0, mt, :], rhs=cur[:],
                                 start=(Xdyn is None), stop=False)
                for j in range(4):
                    nc.tensor.matmul(ps[:], lhsT=WDGt[:, j + 1, mt, :],
                                     rhs=sbY[j][:], start=False, stop=(j == 3))
                zr.append(ps)
            col = t * 8 + b
            z = trz.tile([128, N], BF16, tag="t", name="z")
            nc.scalar.activation(z[:], zr[0][:], AF.Sigmoid, bias=ycZ[:, col:col + 1])
            r = trz.tile([128, N], BF16, tag="t", name="r")
            nc.scalar.activation(r[:], zr[1][:], AF.Sigmoid, bias=ycR[:, col:col + 1])
            zht = trz.tile([128, N], BF16, tag="t", name="zh")
            nc.vector.tensor_mul(zht[:], z[:], cur[:])
            for kt in range(NT):
                pool_k = psE if (DEC_SPREAD and kt % 2 == 1) else psC
                pst = pool_k.tile([128, 128], BF16,
                                  tag="e" if pool_k is psE else "c", name="pst_zd")
                nc.tensor.transpose(pst[:], zht[:, kt * 128:(kt + 1) * 128], ident[:])
                evac(ZHn[:, kt, bsl], pst[:])
            return r, zht

        def phase_b(b, r, zht):
            bsl = slice(b * 128, (b + 1) * 128)
            cur = hdT[b]
            col = t * 8 + b
            sbU = []
            for j in range(4):
                pool_j = (psE if j == 3 else psD if j == 2 else psA) \
                    if DEC_CONV_SPREAD else psA
                ps = pool_j.tile(
                    [128, N], F32,
                    tag="e" if pool_j is psE else "d" if pool_j is psD else "a",
                    name="psUd")
                for kt in range(NT):
                    nc.tensor.matmul(ps[:], lhsT=ZHn[:, kt, bsl], rhs=AT[j][:, kt, :],
                                     start=(kt == 0), stop=(kt == 3))
                sb = yb.tile([128, N], BF16, tag="yb", name="sbU")
                evac(sb[:], ps[:], dve=True)
                sbU.append(sb)
            ps = psB.tile([128, N], F32, tag="b")
            if Xdyn is not None:
                nc.tensor.matmul(ps[:], lhsT=XDWt[0:40, b, 2, :],
                                 rhs=Xdyn[0:40, :], start=True, stop=False)
            nc.tensor.matmul(ps[:], lhsT=WDUt[:, 0, :], rhs=zht[:],
                             start=(Xdyn is None), stop=False)
            for j in range(4):
                nc.tensor.matmul(ps[:], lhsT=WDUt[:, j + 1, :], rhs=sbU[j][:],
                                 start=False, stop=(j == 3))
            hc = trz.tile([128, N], BF16, tag="t", name="hc")
            nc.scalar.activation(hc[:], ps[:], AF.Tanh, bias=ycHC[:, col:col + 1])
            tmp = trz.tile([128, N], BF16, tag="t", name="tmp")
            nc.vector.tensor_sub(tmp[:], cur[:], hc[:])
            nc.vector.tensor_mul(tmp[:], r[:], tmp[:])
            nh = hdTp.tile([128, N], BF16, tag="hdT", name="nh")
            nc.vector.tensor_add(nh[:], hc[:], tmp[:])
            new_hdT.append(nh)

        if PHASE_MAJOR:
            for g in range(B // GSZ):
                grp = list(range(g * GSZ, (g + 1) * GSZ))
                astate = [phase_a(b) for b in grp]
                for bi, b in enumerate(grp):
                    phase_b(b, *astate[bi])
        else:
            for b in range(B):
                phase_b(b, *phase_a(b))
        # go^T = pW . h_b for all b: 8 accumulating matmuls with row-masked
        # copies of pW stack the per-batch rows into one (B, N) psum --
        # streams 512 cols each instead of 32 single-column matmuls
        psgo = psD.tile([B, N], F32, tag="d", name="psgoT")
        for b in range(B):
            nc.tensor.matmul(psgo[:], lhsT=pWm[:, b * 8:(b + 1) * 8],
                             rhs=new_hdT[b][:], start=(b == 0), stop=(b == 7))
        goT = sm.tile([B, N], BF16, tag="goT", bufs=2, name="goT")
        nc.scalar.activation(goT[:], psgo[:], AF.Copy, bias=pb)
        psgt = psC.tile([128, NT * B], BF16, tag="c", name="psgt")
        for kt in range(NT):
            nc.tensor.transpose(psgt[:, kt * 8:(kt + 1) * 8],
                                goT[0:8, kt * 128:(kt + 1) * 128],
                                ident[0:8, 0:8])
        evac(GO_ALL[:, t, :, :], psgt[:])
        if newHdn is not None:
            for kt in range(NT):
                for g2 in range(2):
                    pool_t = (psE, psC)[(kt * 2 + g2) % 2] if SPREAD2 else psE
                    ps = pool_t.tile([128, N], BF16,
                                     tag="e" if pool_t is psE else "c",
                                     name="ps_hd2")
                    for bi in range(4):
                        b = g2 * 4 + bi
                        nc.tensor.transpose(ps[:, bi * 128:(bi + 1) * 128],
                                            new_hdT[b][:, kt * 128:(kt + 1) * 128],
                                            ident[:])
                    evac(newHdn[:, kt, g2 * N:(g2 + 1) * N], ps[:])
        if t < DEC_STEPS - 1:
            newXdyn = xdp.tile([40, N], BF16, tag="xdyn")
            nc.sync.dma_start(newXdyn[0:8, :], goT[:])
            for j in range(4):
                psx = psD.tile([B, N], F32, tag="d")
                for kt in range(NT):
                    nc.tensor.matmul(psx[:], lhsT=GO_ALL[:, t, kt, :],
                                     rhs=AT[j][:, kt, :],
                                     start=(kt == 0), stop=(kt == 3))
                xstg = sm.tile([B, N], BF16, tag="xstg", name="xstg")
                nc.vector.tensor_copy(xstg[:], psx[:])
                nc.sync.dma_start(newXdyn[8 * (j + 1):8 * (j + 2), :], xstg[:])
            Xdyn = newXdyn
        hdT = new_hdT
        Hdn = newHdn

    # ---------------- output: raw bf16 dump ----------------
    nc.sync.dma_start(out8_d[:], GO_ALL[:].rearrange("p a b c -> p (a b c)"))

    for p in reversed(ctxs):
        p.__exit__(None, None, None)


def _build(W):
    nc = bacc.Bacc("TRN2", target_bir_lowering=False, debug=False, num_devices=8)
    dram = {}
    for k, v in W.items():
        if isinstance(v, np.ndarray):
            dt = F8 if k == "AT8H" else BF16
            dram[k] = nc.dram_tensor(k, list(v.shape), dt, kind="ExternalInput")
    dram["ycv"] = nc.dram_tensor("ycv", [8, B * H], BF16, kind="ExternalInput")
    dram["xftb"] = nc.dram_tensor("xftb", [48, L * N], BF16, kind="ExternalInput")
    out8_d = nc.dram_tensor("out8", [128, H * B * NT], BF16,
                            kind="ExternalOutput")

    with tile.TileContext(nc) as tc:
        _emit(nc, tc, dram, out8_d, W["pb"])
    nc.compile()
    return nc


def kernel(x, adj, targets, targets_time, index, Memory, Wq, We1, We2,
           enc_gate_W, enc_gate_b, enc_upd_W, enc_upd_b,
           dec_gate_W, dec_gate_b, dec_upd_W, dec_upd_b, proj_W, proj_b):
    f = lambda a: np.asarray(a, np.float32)
    x = f(x)
    targets_time = f(targets_time)
    W = _pack_weights(f(Memory), f(Wq), f(We1), f(We2),
                      f(enc_gate_W), f(enc_gate_b), f(enc_upd_W), f(enc_upd_b),
                      f(dec_gate_W), f(dec_gate_b), f(dec_upd_W), f(dec_upd_b),
                      f(proj_W), f(proj_b))
    nc = _build(W)

    bf = ml_dtypes.bfloat16
    f8 = ml_dtypes.float8_e4m3
    wmaps = {k: np.ascontiguousarray(
                 np.asarray(v, np.float32).astype(f8 if k == "AT8H" else bf))
             for k, v in W.items() if isinstance(v, np.ndarray)}
    # host-side x-conv features: S_j @ x_b for the 4 adaptive supports
    S_list = [np.ascontiguousarray(W["ATB"][j * N:(j + 1) * N].T)
              for j in range(4)]
    in_maps = []
    for c in range(8):
        xs = x[c * B:(c + 1) * B]                        # (8, 1, 512, 12)
        ycs = targets_time[c * B:(c + 1) * B]            # (8, 5, 1, 12)
        ycv = np.zeros((8, B * H), np.float32)
        ycv[0:5] = ycs[:, :, 0, :].transpose(1, 2, 0).reshape(5, H * B)
        ycv[5] = 1.0
        # xftb rows: slot s=0 is x itself, s=1..4 the support convs S_j @ x;
        # row [s*8+b, t*N+n] = feat[b, n, t]; row 40 = bias ones
        xftb = np.zeros((48, L * N), np.float32)
        xftb[0:8] = xs[:, 0, :, :].transpose(0, 2, 1).reshape(8, L * N)
        for j in range(4):
            Y = np.matmul(S_list[j][None], xs[:, 0, :, :])    # (8, N, L)
            xftb[(j + 1) * 8:(j + 2) * 8] = Y.transpose(0, 2, 1).reshape(8, L * N)
        xftb[40] = 1.0
        in_maps.append({**wmaps, "ycv": ycv.astype(bf), "xftb": xftb.astype(bf)})

    global _LAST_NC, _LAST_INMAPS
    _LAST_NC, _LAST_INMAPS = nc, in_maps
    res = run_bass_kernel_spmd(nc, in_maps, core_ids=list(range(8)))
    outs = []
    for c in range(8):
        raw = res.results[c]["out8"]
        q = np.asarray(raw, np.float32).reshape(128, H, NT, B)
        # out[b, 0, kt*128 + p, t] = q[p, t, kt, b]
        oc = q.transpose(3, 2, 0, 1).reshape(B, N, H)
        outs.append(oc[:, None, :, :])
    return np.concatenate(outs, axis=0).astype(np.float32)


_LAST_NC = None
_LAST_INMAPS = None

